# revision 18
# baseline (speedup 1.0000x reference)
"""GATv2 (3-layer) Trainium2 Bass kernel, 8-core SPMD — v2.

Strategy
--------
- Nodes sharded 2500/core; edges (incl self-loops) sorted by dst, sharded by
  dst range; per-block (128 dst nodes) edge tiles of 128 edges.
- Single-pass softmax: out_i = (sum_j w_ij xl_j) / (sum_j w_ij), so no
  per-edge alpha normalization pass.
- GATv2 logit: att.LeakyRelu(z), z = xl[src]+xr[dst].  With LReLU(z) =
  0.6 z + 0.4|z|:  logit = 0.6(att.z) + 0.4 sum_c |att_c||z_c| sign(att_c).
  Host folds 0.4|att_c| into the Wl/Wr columns (z'' = 0.4|att| z) and
  permutes channels per head into [pos | neg] groups (padded to G each), so
  logit = lin + sum|z''|_pos - sum|z''|_neg, where lin (= 0.6 att.z) comes
  from H extra precomputed columns.  The aggregation output is descaled by
  1/(0.4|att|) in the epilogue (transposed, per-partition scale); channel
  permutation is undone by permuting the NEXT layer's weight rows.
- Per block: one multi-offset indirect gather (xl of all T*128 edges), a
  batched one-hot (de16) build, per-tile z matmuls (de16@xr + I@xl -> PSUM),
  DVE strided reduce with apply_absolute_value direct from PSUM, batched
  block-level logits+exp, then scatter phase: per head one-hot*w built in one
  fused tensor_scalar, PSUM-accumulated u/d matmuls. Epilogue: u*recip(d),
  transpose, Relu(x*sinv + bias) -> feature-major hT in DRAM (padded layout).
- ACT only uses Copy/Exp/Relu/Identity (one activation table, no reloads).
"""
import sys
if '/opt/trn_rl_repo' not in sys.path:
    sys.path.insert(0, '/opt/trn_rl_repo')

from dataclasses import dataclass, field
import numpy as np

import concourse.bass as bass
import concourse.bacc as bacc
import concourse.tile as tile
from concourse import mybir
from concourse import bass_utils
from concourse.masks import make_identity

P = 128
F32 = mybir.dt.float32
F16 = mybir.dt.float16
I32 = mybir.dt.int32

EXP_SHIFT = 4.0  # logits in [-6.1, 5.4]; any constant is exact math-wise


@dataclass(frozen=True)
class LayerCfg:
    f_in: int      # effective (padded) input features
    heads: int
    out_ch: int
    G: int         # padded per-sign group size

    @property
    def w_abs(self):
        return self.heads * 2 * self.G

    @property
    def wg(self):
        return self.w_abs + self.heads


@dataclass
class GatCfg:
    n_cores: int = 8
    shard: int = 2500
    T: int = 18
    layers: tuple = ()
    edge_mode: str = 'full'      # full | noedge
    absmode: tuple = ('act', 'act', 'act')  # per-layer: act | dve
    ag_mode: str = 'collective'  # collective | copy (single-core sim)

    @property
    def n_nodes(self):
        return self.n_cores * self.shard

    @property
    def nblk(self):
        return (self.shard + P - 1) // P

    @property
    def f_final(self):
        return self.layers[-1].w_abs


def _chunks(total, step, start=0):
    out = []
    off = start
    while off < total:
        sz = min(step, total - off)
        out.append((off, sz))
        off += sz
    return out


def _head_chunks(L):
    """Matmul N-chunks for u accumulation: head ranges split on the 512 grid."""
    out = []
    GG = 2 * L.G
    for h in range(L.heads):
        lo, hi = h * GG, (h + 1) * GG
        off = lo
        while off < hi:
            nxt = min(hi, ((off // 512) + 1) * 512)
            out.append((h, off, nxt - off))
            off = nxt
    return out


def build_gat(cfg: GatCfg):
    nc = bacc.Bacc("TRN2", target_bir_lowering=False, debug=False,
                   num_devices=cfg.n_cores)
    NB, T, SH = cfg.nblk, cfg.T, cfg.shard

    # ---------------- external tensors (per-core) ----------------
    srcs = nc.dram_tensor("srcs", [NB, P, T], I32, kind="ExternalInput").ap()
    dst32 = nc.dram_tensor("dst32", [NB, P, T], F32, kind="ExternalInput").ap()
    de16d = nc.dram_tensor("de16d", [NB, P, T * P], F16, kind="ExternalInput").ap()
    xT = nc.dram_tensor("xT", [cfg.layers[0].f_in, SH], F16, kind="ExternalInput").ap()

    wl_d, wr_d, sinvT_d, biasT_d = [], [], [], []
    for li, L in enumerate(cfg.layers):
        nkc = len(_chunks(L.w_abs, P))
        wl_d.append(nc.dram_tensor(f"wl{li}", [L.f_in, L.wg], F16, kind="ExternalInput").ap())
        wr_d.append(nc.dram_tensor(f"wr{li}", [L.f_in, L.wg], F16, kind="ExternalInput").ap())
        sinvT_d.append(nc.dram_tensor(f"sinvT{li}", [P, nkc], F32, kind="ExternalInput").ap())
        biasT_d.append(nc.dram_tensor(f"biasT{li}", [P, nkc], F32, kind="ExternalInput").ap())
    nkf = len(_chunks(cfg.f_final, P))
    wf = nc.dram_tensor("wf", [P, nkf], F16, kind="ExternalInput").ap()
    bf_col = nc.dram_tensor("bf_col", [P, 1], F32, kind="ExternalInput").ap()

    out = nc.dram_tensor("out", [SH, 1], F32, kind="ExternalOutput").ap()

    with tile.TileContext(nc) as tc:
        with tc.tile_pool(name="const", bufs=1) as constp, \
             tc.tile_pool(name="wpool", bufs=1) as wpool, \
             tc.tile_pool(name="sb", bufs=4) as sb, \
             tc.tile_pool(name="blk", bufs=3) as blk, \
             tc.tile_pool(name="psz", bufs=2, space="PSUM") as psz, \
             tc.tile_pool(name="psu", bufs=1, space="PSUM") as psu, \
             tc.tile_pool(name="pss", bufs=1, space="PSUM") as pss, \
             tc.tile_pool(name="ps2", bufs=1, space="PSUM") as ps2, \
             tc.tile_pool(name="dram", bufs=1, space="DRAM") as dram:

            # ---------------- constants ----------------
            ident32 = constp.tile([P, P], F32, name="ident32")
            make_identity(nc, ident32[:])
            ident16 = constp.tile([P, P], F16, name="ident16")
            nc.vector.tensor_copy(out=ident16[:], in_=ident32[:])
            iota_i = constp.tile([P, P], I32, name="iota_i")
            nc.gpsimd.iota(iota_i[:], pattern=[[1, P]], base=0, channel_multiplier=0)
            iota16 = constp.tile([P, P], F16, name="iota16")
            nc.vector.tensor_copy(out=iota16[:], in_=iota_i[:])
            ones16 = constp.tile([P, 1], F16, name="ones16")
            nc.gpsimd.memset(ones16[:], 1.0)
            shift_col = constp.tile([P, 1], F32, name="shift_col")
            nc.gpsimd.memset(shift_col[:], -EXP_SHIFT)

            # resident weights / sinvT / biasT
            wl_sb, wr_sb, sinvT_sb, biasT_sb = [], [], [], []
            for li, L in enumerate(cfg.layers):
                wlk, wrk = [], []
                for ki, (ko, ks) in enumerate(_chunks(L.f_in, P)):
                    t1 = wpool.tile([ks, L.wg], F16, name=f"wl{li}k{ki}")
                    nc.sync.dma_start(out=t1[:], in_=wl_d[li][ko:ko + ks, :])
                    wlk.append(t1)
                    t2 = wpool.tile([ks, L.wg], F16, name=f"wr{li}k{ki}")
                    nc.sync.dma_start(out=t2[:], in_=wr_d[li][ko:ko + ks, :])
                    wrk.append(t2)
                wl_sb.append(wlk)
                wr_sb.append(wrk)
                nkc = len(_chunks(L.w_abs, P))
                ts = wpool.tile([P, nkc], F32, name=f"sinvT{li}")
                nc.sync.dma_start(out=ts[:], in_=sinvT_d[li][:])
                sinvT_sb.append(ts)
                tb = wpool.tile([P, nkc], F32, name=f"biasT{li}")
                nc.sync.dma_start(out=tb[:], in_=biasT_d[li][:])
                biasT_sb.append(tb)
            wf_sb = wpool.tile([P, nkf], F16, name="wf_sb")
            nc.sync.dma_start(out=wf_sb[:], in_=wf[:])
            bf_sb = wpool.tile([P, 1], F32, name="bf_sb")
            nc.sync.dma_start(out=bf_sb[:], in_=bf_col[:])

            hTb = []
            for li, L in enumerate(cfg.layers):
                hTb.append([dram.tile([L.w_abs, min(P, SH - b * P)], F16,
                                      name=f"hT{li}b{b}") for b in range(NB)])

            # =========================================================
            def gemm_alloc(li):
                L = cfg.layers[li]
                ag_in = dram.tile([SH, L.wg], F16, name=f"ag_in{li}")
                xr_sh = dram.tile([NB * P, L.wg], F16, name=f"xr{li}")
                pad = NB * P - SH
                if pad:
                    ztile = sb.tile([pad, L.wg], F16, name="zpad", tag="zpad", bufs=1)
                    nc.gpsimd.memset(ztile[:], 0.0)
                    nc.sync.dma_start(out=xr_sh[SH:NB * P, :], in_=ztile[:])
                return ag_in, xr_sh

            def gemm_block(li, m, ag_in, xr_sh):
                L = cfg.layers[li]
                kcs = _chunks(L.f_in, P)
                ncs = _chunks(L.wg, 512)
                mo = m * P
                mn = min(P, SH - mo)
                lhs = []
                for ki, (ko, ks) in enumerate(kcs):
                    lt = sb.tile([ks, P], F16, name="lhsT", tag=f"lhsT{ki}")
                    if li == 0:
                        nc.sync.dma_start(out=lt[:, :mn], in_=xT[ko:ko + ks, mo:mo + mn])
                    else:
                        nc.sync.dma_start(out=lt[:, :mn],
                                          in_=hTb[li - 1][m][ko:ko + ks, :mn])
                    lhs.append(lt)
                for wsb, dst_d in ((wl_sb[li], ag_in), (wr_sb[li], xr_sh)):
                    og = sb.tile([P, L.wg], F16, name="og", tag="og")
                    for (no, ns) in ncs:
                        pg = ps2.tile([P, 512], F32, name="pg", tag="mm512")
                        for ki in range(len(kcs)):
                            nc.tensor.matmul(
                                out=pg[:mn, :ns],
                                lhsT=lhs[ki][:, :mn],
                                rhs=wsb[ki][:, no:no + ns],
                                start=(ki == 0), stop=(ki == len(kcs) - 1))
                        nc.scalar.copy(out=og[:mn, no:no + ns], in_=pg[:mn, :ns])
                    nc.sync.dma_start(out=dst_d[mo:mo + mn, :], in_=og[:mn, :])

            def final_block(m):
                kcs = _chunks(cfg.f_final, P)
                mo = m * P
                mn = min(P, SH - mo)
                pf = ps2.tile([P, 512], F32, name="pf", tag="mm512")
                lhs = []
                for ki, (ko, ks) in enumerate(kcs):
                    lt = sb.tile([ks, P], F16, name="lhsTf", tag=f"lhsTf{ki}")
                    nc.sync.dma_start(out=lt[:, :mn], in_=hTb[-1][m][ko:ko + ks, :mn])
                    lhs.append(lt)
                for ki, (ko, ks) in enumerate(kcs):
                    nc.tensor.matmul(out=pf[:mn, 0:1], lhsT=lhs[ki][:, :mn],
                                     rhs=wf_sb[:ks, ki:ki + 1],
                                     start=(ki == 0), stop=(ki == len(kcs) - 1))
                of = sb.tile([P, 1], F32, name="of", tag="of")
                nc.scalar.activation(out=of[:mn, :], in_=pf[:mn, 0:1],
                                     func=mybir.ActivationFunctionType.Identity,
                                     bias=bf_sb[:mn, :], scale=1.0)
                nc.sync.dma_start(out=out[mo:mo + mn, :], in_=of[:mn, :])

            # =========================================================
            def edge_phase(li, xl_full, xr_sh, post_block=None):
                L = cfg.layers[li]
                H, G = L.heads, L.G
                WA, WG = L.w_abs, L.wg
                zcs = _chunks(WA, 512)
                ucs = _head_chunks(L)

                def phase1(b):
                    src_i = blk.tile([P, T], I32, name="src_i", tag="src_i")
                    nc.sync.dma_start(out=src_i[:], in_=srcs[b, :, :])
                    de16_all = blk.tile([P, T * P], F16, name="de16_all", tag="de16_all")
                    nc.sync.dma_start(out=de16_all[:], in_=de16d[b, :, :])
                    xr_blk = blk.tile([P, WG], F16, name="xr_blk", tag="xr_blk")
                    nc.sync.dma_start(out=xr_blk[:], in_=xr_sh[b * P:(b + 1) * P, :])

                    xl_all = blk.tile([P, T * WG], F16, name="xl_all", tag="xl_all")
                    for t in range(T):
                        nc.gpsimd.indirect_dma_start(
                            out=xl_all[:, t * WG:(t + 1) * WG],
                            out_offset=None, in_=xl_full[:],
                            in_offset=bass.IndirectOffsetOnAxis(
                                ap=src_i[:, t:t + 1], axis=0))

                    z_lin = pss.tile([P, T * H], F32, name="z_lin", tag="z_lin")
                    red_all = blk.tile([P, T * 2 * H], F32, name="red_all", tag="red_all")

                    for t in range(T):
                        de = de16_all[:, t * P:(t + 1) * P]
                        xlg = xl_all[:, t * WG:(t + 1) * WG]
                        z = psz.tile([P, WA], F32, name="z", tag="z")
                        for (no, ns) in zcs:
                            nc.tensor.matmul(out=z[:, no:no + ns], lhsT=de,
                                             rhs=xr_blk[:, no:no + ns],
                                             start=True, stop=False)
                        nc.tensor.matmul(out=z_lin[:, t * H:(t + 1) * H], lhsT=de,
                                         rhs=xr_blk[:, WA:WG], start=(t == 0),
                                         stop=False)
                        for (no, ns) in zcs:
                            nc.tensor.matmul(out=z[:, no:no + ns], lhsT=ident16[:],
                                             rhs=xlg[:, no:no + ns],
                                             start=False, stop=True)
                        nc.tensor.matmul(out=z_lin[:, t * H:(t + 1) * H], lhsT=ident16[:],
                                         rhs=xlg[:, WA:WG], start=False,
                                         stop=(t == T - 1))
                        red_out = red_all[:, t * 2 * H:(t + 1) * 2 * H] \
                            .rearrange("p (h two) -> p h two", h=H)
                        if cfg.absmode[li] == 'act':
                            labs = sb.tile([P, WA], F16, name="labs", tag="labs", bufs=5)
                            nc.scalar.activation(out=labs[:], in_=z[:],
                                                 func=mybir.ActivationFunctionType.Abs)
                            nc.vector.tensor_reduce(
                                out=red_out,
                                in_=labs[:].rearrange("p (h two g) -> p h two g",
                                                      h=H, two=2),
                                axis=mybir.AxisListType.X, op=mybir.AluOpType.add)
                        else:
                            nc.vector.tensor_reduce(
                                out=red_out,
                                in_=z[:].rearrange("p (h two g) -> p h two g",
                                                   h=H, two=2),
                                axis=mybir.AxisListType.X, op=mybir.AluOpType.add,
                                apply_absolute_value=True)

                    # block-level: logits + exp
                    tdiff = sb.tile([P, T * H], F32, name="tdiff", tag="tdiff")
                    rv = red_all[:].rearrange("p (x two) -> p x two", two=2)
                    nc.vector.tensor_tensor(
                        out=tdiff[:].rearrange("p (x o) -> p x o", o=1),
                        in0=rv[:, :, 0:1], in1=rv[:, :, 1:2],
                        op=mybir.AluOpType.subtract)
                    logits = sb.tile([P, T * H], F32, name="logits", tag="logits")
                    nc.vector.tensor_tensor(out=logits[:], in0=tdiff[:], in1=z_lin[:],
                                            op=mybir.AluOpType.add)
                    w32 = blk.tile([P, T * H], F32, name="w32", tag="w32")
                    nc.scalar.activation(out=w32[:], in_=logits[:],
                                         func=mybir.ActivationFunctionType.Exp,
                                         bias=shift_col[:])
                    dst32_t = blk.tile([P, T], F32, name="dst32_t", tag="dst32_t")
                    nc.sync.dma_start(out=dst32_t[:], in_=dst32[b, :, :])
                    return xl_all, w32, dst32_t

                def phase2(b, xl_all, w32, dst32_t):
                    bn = min(P, SH - b * P)
                    u_ps = psu.tile([P, WA + H], F32, name="u_ps", tag="u_ps")
                    d_ps = u_ps[:, WA:WA + H]
                    # emission order of (col, width) incl. the per-head d column;
                    # one PSUM-zeroing start and one stop per 2KB region
                    mm_list = []
                    for h in range(H):
                        mm_list += [(h, no, ns) for (hh, no, ns) in ucs if hh == h]
                        mm_list.append((h, WA + h, 1))
                    regions = [(no * 4) // 2048 for (_, no, ns) in mm_list]
                    first_i = {}
                    last_i = {}
                    for i, r in enumerate(regions):
                        first_i.setdefault(r, i)
                        last_i[r] = i
                    for t in range(T):
                        ohw_t = {}
                        for h in range(H):
                            ohw = sb.tile([P, P], F16, name="ohw", tag="ohw", bufs=8)
                            nc.vector.tensor_scalar(
                                out=ohw[:], in0=iota16[:],
                                scalar1=dst32_t[:, t:t + 1],
                                scalar2=w32[:, t * H + h:t * H + h + 1],
                                op0=mybir.AluOpType.is_equal,
                                op1=mybir.AluOpType.mult)
                            ohw_t[h] = ohw
                        for i, (h, no, ns) in enumerate(mm_list):
                            rhs = (ones16[:] if no >= WA else
                                   xl_all[:, t * WG + no:t * WG + no + ns])
                            nc.tensor.matmul(
                                out=u_ps[:, no:no + ns], lhsT=ohw_t[h][:], rhs=rhs,
                                start=(t == 0 and i == first_i[regions[i]]),
                                stop=(t == T - 1 and i == last_i[regions[i]]))
                    # epilogue
                    dsb = sb.tile([P, H], F32, name="dsb", tag="dsb")
                    nc.vector.tensor_scalar(out=dsb[:], in0=d_ps, scalar1=1e-30,
                                            scalar2=None, op0=mybir.AluOpType.add)
                    recip = sb.tile([P, H], F32, name="recip", tag="recip")
                    nc.vector.reciprocal(out=recip[:], in_=dsb[:])
                    u_sb = sb.tile([P, WA], F32, name="u_sb", tag="u_sb")
                    for (h, no, ns) in ucs:
                        nc.scalar.activation(
                            out=u_sb[:, no:no + ns], in_=u_ps[:, no:no + ns],
                            func=mybir.ActivationFunctionType.Copy,
                            scale=recip[:, h:h + 1])
                    for kc, (fo, fs) in enumerate(_chunks(WA, P)):
                        tp2 = ps2.tile([P, 512], F32, name="tp2", tag="mm512")
                        nc.tensor.transpose(out=tp2[:fs, :P], in_=u_sb[:, fo:fo + fs],
                                            identity=ident32[:])
                        hts = sb.tile([P, P], F16, name="hts", tag="hts")
                        nc.scalar.activation(out=hts[:fs, :bn], in_=tp2[:fs, :bn],
                                             func=mybir.ActivationFunctionType.Relu,
                                             bias=biasT_sb[li][:fs, kc:kc + 1],
                                             scale=sinvT_sb[li][:fs, kc:kc + 1])
                        nc.sync.dma_start(
                            out=hTb[li][b][fo:fo + fs, :bn],
                            in_=hts[:fs, :bn])

                # 1-block software pipeline: ph1(b+1) is emitted before ph2(b);
                # post_block(b) interleaves next-layer gemm / final linear
                prev = None
                for b in range(NB):
                    cur = phase1(b)
                    if prev is not None:
                        phase2(b - 1, *prev)
                        if post_block is not None:
                            post_block(b - 1)
                    prev = cur
                phase2(NB - 1, *prev)
                if post_block is not None:
                    post_block(NB - 1)

            # =========================================================
            def run_ag(li, ag_in):
                L = cfg.layers[li]
                if cfg.ag_mode == 'collective':
                    xl_full = dram.tile([cfg.n_nodes, L.wg], F16, name=f"xl_full{li}",
                                        addr_space="Shared")
                    nc.gpsimd.collective_compute(
                        "AllGather", mybir.AluOpType.bypass,
                        replica_groups=[list(range(cfg.n_cores))],
                        ins=[ag_in[:]], outs=[xl_full[:]])
                else:
                    xl_full = dram.tile([cfg.n_nodes, L.wg], F16, name=f"xl_full{li}")
                    for r in range(cfg.n_nodes // SH):
                        nc.sync.dma_start(out=xl_full[r * SH:(r + 1) * SH, :], in_=ag_in[:])
                return xl_full

            ag0, xr0 = gemm_alloc(0)
            for m in range(NB):
                gemm_block(0, m, ag0, xr0)
            xl_full = run_ag(0, ag0)
            xr_cur = xr0
            for li in range(len(cfg.layers)):
                if li + 1 < len(cfg.layers):
                    ag_nxt, xr_nxt = gemm_alloc(li + 1)
                    pb = (lambda b, _li=li, _ag=ag_nxt, _xr=xr_nxt:
                          gemm_block(_li + 1, b, _ag, _xr))
                else:
                    ag_nxt = xr_nxt = None
                    pb = final_block
                if cfg.edge_mode != 'noedge':
                    edge_phase(li, xl_full, xr_cur, post_block=pb)
                else:
                    for m in range(NB):
                        pb(m)
                if li + 1 < len(cfg.layers):
                    xl_full = run_ag(li + 1, ag_nxt)
                    xr_cur = xr_nxt

    nc.compile()
    return nc


# =====================================================================
# host-side data prep
# =====================================================================

def prep_layers(inputs):
    """Per-layer padded layouts + augmented weights (shared by all cores)."""
    metas = []     # LayerCfg list
    tensors = []   # per layer dict: wl, wr, sinvT, biasT
    prev_map = None
    heads_out = [(3, 64), (3, 256), (1, 512)]
    for li in range(3):
        H, C = heads_out[li]
        Wl = np.asarray(inputs[f'Wl{li + 1}'], np.float32)
        Wr = np.asarray(inputs[f'Wr{li + 1}'], np.float32)
        att = np.asarray(inputs[f'att{li + 1}'], np.float32)  # [H, C]
        bias = np.asarray(inputs[f'b{li + 1}'], np.float32).reshape(-1)
        pos_idx = [np.where(att[h] >= 0)[0] for h in range(H)]
        neg_idx = [np.where(att[h] < 0)[0] for h in range(H)]
        G = max(1, max(max(len(p), len(n)) for p, n in zip(pos_idx, neg_idx)))
        WA = H * 2 * G
        WG = WA + H
        # permute rows to previous layer's padded layout
        if prev_map is None:
            Wl_r, Wr_r = Wl, Wr
            F_eff = Wl.shape[0]
        else:
            F_eff = len(prev_map)
            valid = prev_map >= 0
            Wl_r = np.zeros((F_eff, Wl.shape[1]), np.float32)
            Wr_r = np.zeros((F_eff, Wr.shape[1]), np.float32)
            Wl_r[valid] = Wl[prev_map[valid]]
            Wr_r[valid] = Wr[prev_map[valid]]
        wl_aug = np.zeros((F_eff, WG), np.float32)
        wr_aug = np.zeros((F_eff, WG), np.float32)
        col_map = np.full(WA, -1, np.int64)
        sinv = np.zeros(WA, np.float32)
        bias_pad = np.zeros(WA, np.float32)
        for h in range(H):
            for grp, base in ((pos_idx[h], h * 2 * G), (neg_idx[h], h * 2 * G + G)):
                for j, c in enumerate(grp):
                    col = base + j
                    v = 0.4 * abs(att[h, c])
                    wl_aug[:, col] = Wl_r[:, h * C + c] * v
                    wr_aug[:, col] = Wr_r[:, h * C + c] * v
                    col_map[col] = h * C + c
                    sinv[col] = 1.0 / v if v > 0 else 0.0
                    bias_pad[col] = bias[h * C + c]
            wl_aug[:, WA + h] = 0.6 * (Wl_r[:, h * C:(h + 1) * C] @ att[h])
            wr_aug[:, WA + h] = 0.6 * (Wr_r[:, h * C:(h + 1) * C] @ att[h])
        nkc = len(_chunks(WA, P))
        sinvT = np.zeros((P, nkc), np.float32)
        biasT = np.zeros((P, nkc), np.float32)
        for kc, (fo, fs) in enumerate(_chunks(WA, P)):
            sinvT[:fs, kc] = sinv[fo:fo + fs]
            biasT[:fs, kc] = bias_pad[fo:fo + fs]
        metas.append(LayerCfg(f_in=F_eff, heads=H, out_ch=C, G=G))
        tensors.append(dict(wl=wl_aug.astype(np.float16), wr=wr_aug.astype(np.float16),
                            sinvT=sinvT, biasT=biasT))
        prev_map = col_map
    # final linear
    WA3 = metas[-1].w_abs
    wf_flat = np.asarray(inputs['Wf'], np.float32).reshape(-1)
    wf_pad = np.zeros(WA3, np.float32)
    valid = prev_map >= 0
    wf_pad[valid] = wf_flat[prev_map[valid]]
    nkf = len(_chunks(WA3, P))
    wfp = np.zeros((P, nkf), np.float32)
    for ki, (ko, ks) in enumerate(_chunks(WA3, P)):
        wfp[:ks, ki] = wf_pad[ko:ko + ks]
    fin = dict(wf=wfp.astype(np.float16),
               bf_col=np.full((P, 1), np.asarray(inputs['bf'], np.float32).reshape(-1)[0],
                              np.float32))
    return metas, tensors, fin


def prep_host(inputs, n_cores=8, shard=2500):
    N = n_cores * shard
    NB = (shard + P - 1) // P
    x = np.asarray(inputs['x'], np.float32)
    ei = np.asarray(inputs['edge_index']).astype(np.int64)
    loop = np.arange(N, dtype=np.int64)
    src = np.concatenate([ei[0], loop])
    dst = np.concatenate([ei[1], loop])
    order = np.argsort(dst, kind='stable')
    src_s, dst_s = src[order], dst[order]

    cnt = np.zeros((n_cores, NB), dtype=np.int64)
    bounds = {}
    for c in range(n_cores):
        for b in range(NB):
            blk_lo = c * shard + b * P
            blk_hi = min(blk_lo + P, (c + 1) * shard)
            lo = np.searchsorted(dst_s, blk_lo)
            hi = np.searchsorted(dst_s, blk_hi)
            bounds[(c, b)] = (lo, hi, blk_lo)
            cnt[c, b] = hi - lo
    T = int((cnt.max() + P - 1) // P)

    metas, ltens, fin = prep_layers(inputs)
    cfg = GatCfg(n_cores=n_cores, shard=shard, T=T, layers=tuple(metas))

    in_maps = []
    for c in range(n_cores):
        srcs = np.zeros((NB, P, T), dtype=np.int32)
        dstl = np.full((NB, P, T), -1, dtype=np.int32)
        for b in range(NB):
            lo, hi, blk_lo = bounds[(c, b)]
            ne = hi - lo
            s = np.zeros(T * P, dtype=np.int32)
            d = np.full(T * P, -1, dtype=np.int32)
            s[:ne] = src_s[lo:hi]
            d[:ne] = (dst_s[lo:hi] - blk_lo)
            srcs[b] = s.reshape(T, P).T
            dstl[b] = d.reshape(T, P).T
        dst32f = dstl.astype(np.float32)
        # de16[b, d, t*128+e] = (dst_local[e, t] == d)   (transposed one-hot)
        dlT = dstl.transpose(0, 2, 1).reshape(NB, 1, T * P)  # value dst(e,t)
        de16 = (np.arange(P, dtype=np.int32).reshape(1, P, 1) == dlT
                ).astype(np.float16)
        xTl = np.ascontiguousarray(x[c * shard:(c + 1) * shard, :].T).astype(np.float16)
        im = {'srcs': srcs, 'dst32': dst32f, 'de16d': de16, 'xT': xTl}
        for li, td in enumerate(ltens):
            im[f'wl{li}'] = td['wl']
            im[f'wr{li}'] = td['wr']
            im[f'sinvT{li}'] = td['sinvT']
            im[f'biasT{li}'] = td['biasT']
        im['wf'] = fin['wf']
        im['bf_col'] = fin['bf_col']
        in_maps.append(im)
    return cfg, in_maps


_CACHE = {}


def kernel(**inputs) -> np.ndarray:
    cfg, in_maps = prep_host(inputs)
    key = (cfg.T, cfg.layers)
    if key not in _CACHE:
        _CACHE[key] = build_gat(cfg)
    nc = _CACHE[key]
    res = bass_utils.run_bass_kernel_spmd(nc, in_maps, core_ids=list(range(cfg.n_cores)))
    out = np.concatenate([res.results[c]['out'] for c in range(cfg.n_cores)], axis=0)
    return out.astype(np.float32)
